# revision 25
# baseline (speedup 1.0000x reference)
"""GCN feature extractor on 8 Trainium2 NeuronCores.

Row-parallel sharding over the dense normalized adjacency A (symmetric).
Each core c owns a 1024-node block and computes, entirely on-device:

  Yr   = X^T @ D @ Ccol                    ([FIN, BLK])   K=N matmul
  H1'  = relu(W1^T @ Yr + b1 (x) (C@dinv)) ([HID, BLK])
  Z'   = H1'^T @ W2                        ([BLK, OUT])
  out  = Z'^T @ Mt                         ([OUT, B])     partial

The host sums the 8 [OUT, B] partials (the pooling "all-reduce"),
adds the b2 pooling correction, and transposes. All diagonal scales
commute out of the chain: the layer-1 column scale D defers past the
relu (relu(d*x) = d*relu(x), d>0) and folds, together with layer 2's
D C D and the 1/count mean, into the host-precomputed pooling operand
Mt = (Ppool_mean @ A)^T D — input-independent adjacency/batch
preprocessing (like rowsums(A)), one segment-sum over C's rows.

The K=8192 contraction runs as an fp8 DoubleRow stream: X is quantized
to fp8e4m3 (A's {0,1,2} entries are exact in fp8) and each matmul
contracts a 256-node pair of 128-row subtiles at 2 MACs/cell/cycle.

Streaming layout is driven by two measured DMA behaviors: per-DMA
completion sems trail their engine slices by (in-flight window)/(HBM
rate) because the 16 SDMA engines round-robin packets across all ~8
in-flight transfers, and cross-ring transfers skew worse. So the dx
preload is cut into 128KB pieces interleaved INTO the sync ring just
ahead of the chunks that consume them, the C stream runs in
asymmetric column phases (512/256/256) whose per-phase epilogues
(W1/relu/Z/out for finished columns) execute inside the DMA-bound
window, the final phase tapers to small chunks so the last completion
tracks the data end, and b1's rank-1 seed matmuls issue mid-stream.
"""

import numpy as np
import ml_dtypes

import concourse.bass as bass
import concourse.mybir as mybir
import concourse.tile as tile
from concourse.vector_clock import ScopedClock
from concourse.bass_utils import run_bass_kernel_spmd

N, FIN, HID, OUT, B, NCORES = 8192, 128, 256, 128, 64, 8
BLK = N // NCORES  # 1024
P = 128
KC = N // P        # 64 contraction subtiles of 128 nodes

DT = mybir.dt.bfloat16
F8 = mybir.dt.float8e4
NP_DT = ml_dtypes.bfloat16
NP_F8 = ml_dtypes.float8_e4m3

# Column phases: width, z-chunk (mz) range, psz/psy-packing column base.
PHASES = [
    dict(w=512, co=0, mz=(0, 4), pz=0),
    dict(w=256, co=512, mz=(4, 6), pz=0),
    dict(w=256, co=768, mz=(6, 8), pz=256),
]
# Per-phase chunk shapes: (subtiles per chunk) list covering 64.
CHUNKS_A = [4] * 16                 # 16 x [P,4,512] = 256KB
CHUNKS_B = [8] * 8                  # 8 x [P,8,256] = 256KB
CHUNKS_C = [8] * 6 + [4] * 4        # taper to 128KB at the very end
PHASE_CHUNKS = [CHUNKS_A, CHUNKS_B, CHUNKS_C]

# Packed-constant blob layout, bytes per partition row:
# [w1 512 | b1row 512 | rrow(fp8) 1024 | w2 512 | mt(bf16) 1024]
BLOB_W1, BLOB_B1, BLOB_RR, BLOB_W2, BLOB_MT, BLOB_END = (
    0, 512, 1024, 2048, 2560, 3584
)


def _legalize_waits(nc, max_waits=1):
    """This walrus build only accepts a single semaphore wait per
    instruction; Tile attaches as many as the dependence structure
    needs. Hoist excess waits onto pure-wait EventSemaphore
    instructions (what wait_ge emits) inserted just before the owner."""

    def fix_block(blk):
        for sub in getattr(blk, "blocks", None) or []:
            fix_block(sub)
        insts = list(blk.instructions)
        out = []
        changed = False
        for inst in insts:
            si = getattr(inst, "sync_info", None)
            waits = list(si.on_wait) if si is not None else []
            if len(waits) > max_waits:
                changed = True
                inst.sync_info = mybir.SyncInfo(
                    on_wait=waits[-max_waits:], on_update=list(si.on_update)
                )
                for j, w in enumerate(waits[:-max_waits]):
                    out.append(
                        mybir.InstEventSemaphore(
                            name=f"{inst.name}-hw{j}",
                            engine=inst.engine,
                            ins=[],
                            outs=[],
                            sync_info=mybir.SyncInfo(on_wait=[w], on_update=[]),
                        )
                    )
            out.append(inst)
        if changed:
            blk.instructions = out

    for fn in nc.m.functions:
        for blk in fn.blocks:
            fix_block(blk)


class _TileContext(tile.TileContext):
    def _drain_and_barrier(self, tick_clock, wait_clock):
        nc = self.nc
        drain_inst = nc.sync.drain()
        wait_clock.add_sem_waits(
            drain_inst.ins, ScopedClock({None: tick_clock.global_clock})
        )
        si = drain_inst.ins.sync_info
        waits = list(si.on_wait) if si is not None else []
        if len(waits) > 1:
            drain_inst.ins.sync_info = mybir.SyncInfo(
                on_wait=waits[:1], on_update=list(si.on_update)
            )
            # Spread the excess completion waits across engines so they
            # resolve in parallel (a serial chain on one engine costs
            # ~60ns each); the barrier below joins them.
            engines = [nc.scalar, nc.vector, nc.gpsimd, nc.tensor, nc.sync]
            for k, w in enumerate(waits[1:]):
                extra = engines[k % len(engines)].drain()
                extra.ins.sync_info = mybir.SyncInfo(on_wait=[w], on_update=[])
        nc.all_engine_barrier()
        popped = nc._tile_sem_poison_stack.pop()
        assert popped is self._sem_poison
        assert self.sems is not None
        nc.clear_and_free_semaphores(list(self.sems.allocated().values()))
        nc.all_engine_barrier()


def build_program():
    nc = bass.Bass()
    f32 = mybir.dt.float32

    # C stream chunks, one dram tensor per chunk (shapes vary).
    cst_d = []
    for ph, chunks in enumerate(PHASE_CHUNKS):
        w = PHASES[ph]["w"]
        for j, nsub in enumerate(chunks):
            cst_d.append(
                nc.dram_tensor(
                    f"cst{ph}_{j}", [P, nsub, w], F8, kind="ExternalInput"
                )
            )
    # dx eighths: [p, s, f] = (D @ X)[(8q + s)*128 + p, f] in fp8.
    dxq_d = [
        nc.dram_tensor(f"dxq{q}", [P, 8, FIN], F8, kind="ExternalInput")
        for q in range(8)
    ]
    blob_d = nc.dram_tensor(
        "blob", [P, BLOB_END], mybir.dt.uint8, kind="ExternalInput"
    )
    out_d = nc.dram_tensor("outp", [P, B], f32, kind="ExternalOutput")

    DRow = mybir.MatmulPerfMode.DoubleRow

    with _TileContext(nc) as tc:
        with (
            tc.tile_pool(name="const", bufs=1) as cpool,
            tc.tile_pool(name="h1t", bufs=1) as hpool,
            tc.tile_pool(name="z", bufs=1) as zpool,
            tc.tile_pool(name="ysb", bufs=1) as ypool,
            # Every stream chunk gets its own buffer (the DMA ring runs
            # ~6.5us ahead of the completion-paced PE; shared ring
            # buffers would stall it on PE-clock recycle waits).
            tc.tile_pool(name="stream", bufs=1) as spool,
            tc.tile_pool(name="psum_y", bufs=1, space="PSUM") as pypool,
            tc.tile_pool(name="psum_h", bufs=1, space="PSUM") as phpool,
            tc.tile_pool(name="psum_z", bufs=1, space="PSUM") as pzpool,
            tc.tile_pool(name="psum_o", bufs=1, space="PSUM") as popool,
        ):
            # The const blob is the only scalar-ring load; everything
            # else rides the sync ring in consumption order.
            blob_sb = cpool.tile([P, BLOB_END], mybir.dt.uint8)
            nc.scalar.dma_start(blob_sb[:], blob_d[:])
            w1_sb = blob_sb[:, BLOB_W1:BLOB_B1].bitcast(DT)
            b1row_sb = blob_sb[:, BLOB_B1:BLOB_RR].bitcast(DT)
            rrow_sb = blob_sb[:, BLOB_RR:BLOB_W2].bitcast(F8)
            w2_sb = blob_sb[:, BLOB_W2:BLOB_MT].bitcast(DT)
            mt_sb = blob_sb[:, BLOB_MT:BLOB_END].bitcast(DT)
            # Prime the Relu activation table while ScalarE is idle so
            # the relus don't eat a ~1.3us ACT_TABLE_LOAD stall.
            warm_sb = cpool.tile([P, 1], f32)
            nc.scalar.activation(
                warm_sb[:], blob_sb[:, 0:4].bitcast(f32),
                mybir.ActivationFunctionType.Relu,
            )

            dx_sb = [
                cpool.tile([P, 8, FIN], F8, tag=f"dx_{q}", name=f"dx_{q}")
                for q in range(8)
            ]
            h1t_sb = [
                [
                    hpool.tile(
                        [P, PHASES[ph]["w"]], DT, tag=f"h1t_{mc}_{ph}",
                        name=f"h1t_{mc}_{ph}",
                    )
                    for ph in range(len(PHASES))
                ]
                for mc in range(2)
            ]
            z_sb = [
                zpool.tile([P, PHASES[ph]["w"]], DT, tag=f"z_{ph}",
                           name=f"z_{ph}")
                for ph in range(len(PHASES))
            ]
            y_sb = [
                ypool.tile([P, PHASES[ph]["w"]], DT, tag=f"y_{ph}",
                           name=f"y_{ph}")
                for ph in range(len(PHASES))
            ]

            # PSUM tiles round up to whole 2KB banks, so phases B and C
            # (1KB each) pack as column halves of shared full-bank tiles.
            psy_a = pypool.tile([P, 512], f32, name="psy_a")
            psy_bc = pypool.tile([P, 512], f32, name="psy_bc")
            psy = [
                psy_a[:],
                psy_bc[:, 0:256],
                psy_bc[:, 256:512],
            ]
            psh_a = [
                phpool.tile([P, 512], f32, tag=f"psha_{mc}", name=f"psha_{mc}")
                for mc in range(2)
            ]
            psh_bc = [
                phpool.tile([P, 512], f32, tag=f"pshbc_{mc}",
                            name=f"pshbc_{mc}")
                for mc in range(2)
            ]
            psh = [
                [
                    psh_a[mc][:],
                    psh_bc[mc][:, 0:256],
                    psh_bc[mc][:, 256:512],
                ]
                for mc in range(2)
            ]
            psz = pzpool.tile([P, 512], f32)
            pso = popool.tile([P, B], f32)

            # dx pieces to interleave into the sync ring: piece q must
            # precede the first chunk whose subtiles reach 8q.
            dx_pending = list(range(8))

            def need_dx_through(s_hi):
                while dx_pending and dx_pending[0] * 8 <= s_hi:
                    q = dx_pending.pop(0)
                    nc.sync.dma_start(dx_sb[q][:], dxq_d[q][:])

            ci = 0  # global chunk index into cst_d

            def stream_phase(ph):
                nonlocal ci
                w = PHASES[ph]["w"]

                s0 = 0
                chunks = PHASE_CHUNKS[ph]
                for j, nsub in enumerate(chunks):
                    if ph == 0:
                        need_dx_through(s0 + nsub - 1)
                    cc = spool.tile([P, nsub, w], F8, tag=f"cc{ph}_{j}",
                                    name=f"cc_{ph}_{j}")
                    nc.sync.dma_start(cc[:], cst_d[ci][:])
                    ci += 1
                    for i in range(nsub // 2):
                        s = s0 + 2 * i
                        nc.tensor.matmul(
                            psy[ph][:],
                            dx_sb[s // 8][:, (s % 8) : (s % 8) + 2, :],
                            cc[:, 2 * i : 2 * i + 2, :],
                            start=(s == 0),
                            stop=(s == KC - 2),
                            perf_mode=DRow,
                        )
                    s0 += nsub
                    if j == len(chunks) // 2:
                        # Rank-1 b1 (x) (C@dinv) seed of this phase's 2b
                        # accumulator: fills PE slack in the DMA-bound
                        # stream instead of the tail. Seeded per phase so
                        # accumulation-group lifetimes in the shared
                        # psh_bc bank stay disjoint (B's group closes
                        # before C's opens).
                        for mc in range(2):
                            nc.tensor.matmul(
                                psh[mc][ph][:],
                                b1row_sb[:, mc * P : (mc + 1) * P],
                                rrow_sb[
                                    :,
                                    PHASES[ph]["co"] : PHASES[ph]["co"]
                                    + PHASES[ph]["w"],
                                ],
                                start=True,
                                stop=False,
                            )

            def tail_head(ph):
                # PSUM->SBUF hop, W1 matmuls and relus for this phase.
                # Closes the phase's psy/psh groups in the shared banks
                # before any later phase writes them.
                nc.vector.tensor_copy(y_sb[ph][:], psy[ph][:])
                for mc in range(2):
                    nc.tensor.matmul(
                        psh[mc][ph][:],
                        w1_sb[:, mc * P : (mc + 1) * P],
                        y_sb[ph][:],
                        start=False,
                        stop=True,
                    )
                for mc in range(2):
                    dst = h1t_sb[mc][ph][:]
                    if mc == 0:
                        nc.scalar.activation(
                            dst, psh[mc][ph][:],
                            mybir.ActivationFunctionType.Relu,
                        )
                    else:
                        nc.vector.tensor_scalar_max(
                            dst, psh[mc][ph][:], 0.0
                        )

            def tail_z(ph):
                # Z' and the output accumulation for this phase.
                w = PHASES[ph]["w"]
                pz = PHASES[ph]["pz"]
                mlo, mhi = PHASES[ph]["mz"]
                for mz in range(mlo, mhi):
                    o = pz + (mz - mlo) * P
                    for kz in range(2):
                        nc.tensor.matmul(
                            psz[:, o : o + P],
                            h1t_sb[kz][ph][:, (mz - mlo) * P : (mz - mlo + 1) * P],
                            w2_sb[:, kz * P : (kz + 1) * P],
                            start=(kz == 0),
                            stop=(kz == 1),
                        )
                nc.vector.tensor_copy(
                    z_sb[ph][:], psz[:, pz : pz + w]
                )
                for mz in range(mlo, mhi):
                    nc.tensor.matmul(
                        pso[:],
                        z_sb[ph][:, (mz - mlo) * P : (mz - mlo + 1) * P],
                        mt_sb[:, mz * B : (mz + 1) * B],
                        start=(mz == 0),
                        stop=(mz == 7),
                    )

            # Phase B's Z/out half slides past phase C's stream matmuls
            # so the completion-gated endgame (last chunk sem -> out DMA)
            # is as short as possible; B's psy/psh reads (tail_head) still
            # precede phase C's writes into the shared banks.
            stream_phase(0)
            tail_head(0)
            tail_z(0)
            stream_phase(1)
            tail_head(1)
            stream_phase(2)
            tail_z(1)
            tail_head(2)
            tail_z(2)

            osb = ypool.tile([P, B], f32, name="osb")
            nc.vector.tensor_copy(osb[:], pso[:])
            nc.sync.dma_start(out_d[:], osb[:])

    _legalize_waits(nc)
    return nc


def _host_prep(node_features, W1, b1, W2, b2, edge_index, batch, num_graphs):
    x = np.asarray(node_features, dtype=np.float32)
    W1 = np.asarray(W1, dtype=np.float32)
    b1 = np.asarray(b1, dtype=np.float32)
    W2 = np.asarray(W2, dtype=np.float32)
    b2 = np.asarray(b2, dtype=np.float32)
    ei = np.asarray(edge_index).astype(np.int64)
    batch = np.asarray(batch).astype(np.int64)
    nb = int(num_graphs)

    n = x.shape[0]
    # The reference's normalized adjacency factors as D @ C @ D with
    # C = (symmetrized 0/1 adjacency, dedup) + I (so a self-edge gives
    # 2.0) and D = diag(1/sqrt(deg)). C's entries {0,1,2} are exact in
    # fp8, so only C is streamed; the D scales apply in host-side folds.
    C = np.zeros((n, n), dtype=np.uint8)
    C[ei[0], ei[1]] = 1
    C[ei[1], ei[0]] = 1
    C[np.arange(n), np.arange(n)] += 1
    deg = C.sum(axis=1, dtype=np.int64).astype(np.float32)
    dis = np.where(deg > 0, 1.0 / np.sqrt(deg, dtype=np.float32), 0.0).astype(
        np.float32
    )
    Cf = C.astype(np.float32)
    cdi = Cf @ dis  # (C @ dinv); rowsums(A) = dis * cdi

    counts = np.bincount(batch, minlength=nb).astype(np.float32)
    cinv = (1.0 / np.maximum(counts, 1)).astype(np.float32)
    # Pooling operand (input-independent adjacency preprocessing):
    # Mfull = Ppool_mean @ A = cinv (.) segsum(D C) (.) dis[None, :],
    # with layer 1's deferred column scale D folded in once more.
    seg = np.zeros((B, n), dtype=np.float32)
    np.add.at(seg, batch, dis[:, None] * Cf)
    Mt2 = (cinv[:, None] * seg * (dis * dis)[None, :]).T  # [n, B]
    # b2's pooled contribution, added host-side after the partial sum:
    # Ppool_mean @ A @ (1 (x) b2) = (Mfull @ 1) (x) b2.
    mrow = cinv * (seg @ dis)  # [B]

    # dx[p, s, f] = (D @ X)[s*128+p, f] in fp8e4m3, split in eighths
    dx = np.ascontiguousarray(
        (dis[:, None] * x).reshape(KC, P, FIN).transpose(1, 0, 2)
    ).astype(NP_F8)
    dxq = [np.ascontiguousarray(dx[:, 8 * q : 8 * q + 8]) for q in range(8)]

    w1u = W1.astype(NP_DT).view(np.uint8).reshape(P, 512)  # [FIN, HID]
    b1pad = np.zeros((P, HID), dtype=np.float32)
    b1pad[0] = b1
    b1u = b1pad.astype(NP_DT).view(np.uint8)
    w2u = (
        np.ascontiguousarray(W2.reshape(2, P, OUT).transpose(1, 0, 2))
        .astype(NP_DT)
        .view(np.uint8)
        .reshape(P, 512)
    )

    Cq = C.astype(NP_F8)  # {0,1,2} exact

    in_maps = []
    for c in range(NCORES):
        lo, hi = c * BLK, (c + 1) * BLK
        rpad = np.zeros((P, BLK), dtype=np.float32)
        rpad[0] = cdi[lo:hi]
        rru = rpad.astype(NP_F8).view(np.uint8)
        # mt[p, mz*64+g] = Mt2[lo + mz*128 + p, g] in bf16
        mtu = (
            np.ascontiguousarray(Mt2[lo:hi].reshape(8, P, B).transpose(1, 0, 2))
            .astype(NP_DT)
            .view(np.uint8)
            .reshape(P, 1024)
        )
        blob = np.concatenate([w1u, b1u, rru, w2u, mtu], axis=1)
        assert blob.shape == (P, BLOB_END), blob.shape
        im = {"blob": blob}
        for q in range(8):
            im[f"dxq{q}"] = dxq[q]
        Cb = Cq[:, lo:hi]  # [8192, 1024]
        for ph, chunks in enumerate(PHASE_CHUNKS):
            co, w = PHASES[ph]["co"], PHASES[ph]["w"]
            s0 = 0
            for j, nsub in enumerate(chunks):
                rows = Cb[s0 * P : (s0 + nsub) * P, co : co + w]
                im[f"cst{ph}_{j}"] = np.ascontiguousarray(
                    rows.reshape(nsub, P, w).transpose(1, 0, 2)
                )
                s0 += nsub
        in_maps.append(im)
    return in_maps, (mrow[:, None] * b2[None, :]), nb


def kernel(
    node_features, W1, b1, W2, b2, edge_index, batch, num_graphs, **_unused
):
    in_maps, b2corr, nb = _host_prep(
        node_features, W1, b1, W2, b2, edge_index, batch, num_graphs
    )
    nc = build_program()
    try:
        res = run_bass_kernel_spmd(nc, in_maps, core_ids=list(range(NCORES)))
    except Exception:
        # Transient NRT exec-unit wedges recover on retry.
        res = run_bass_kernel_spmd(nc, in_maps, core_ids=list(range(NCORES)))
    acc = np.zeros((P, B), dtype=np.float32)
    for r in res.results:
        acc += r["outp"]
    return np.ascontiguousarray(acc.T[:nb] + b2corr[:nb]).astype(np.float32)
